# revision 7
# baseline (speedup 1.0000x reference)
"""MoE routed-expert kernel for Trainium2 (8 NeuronCores, SPMD).

Problem: N=16384 tokens, D=768, H=768, C=2, E=20 experts.
  y[n] = relu(x[n] @ W1[e] + b1[e]) @ W2[e] + b2[e],  e = component_idx[n]

Strategy
--------
Host side (numpy): sort tokens by expert; search cap triples (c0,c1,c2)
minimizing padded tokens R=c0+c1+c2 such that the 20 expert groups can be
split into <=24 pieces that fill 8 cores x 3 "expert slots" (slot s holds
c_s tokens of ONE expert per core).  Every core runs the SAME static
program (SPMD); which expert a slot holds is purely which weights/tokens
the host stages into that core's buffers.

Device side (Bass/Tile, per core): x is staged per chunk as [128, DT*sz]
(d-tiles contiguous) so ONE DMA feeds a whole chunk.  DMA issues are split
across the Sync and Scalar queues (descriptor issue costs ~0.6us each on
one queue).  The first chunk is small (256 tokens) and runs dt-major so
the PE starts as soon as w1_d0 + one 0.4MB chunk land.  Layer 1: 6x6
accumulating fp16 matmuls per chunk; relu+bias fused on ScalarE.  Layer 2:
either 6 plain [128,2] matmuls (L2_PACK=0) or 4-way PE column groups +
selector reduction (L2_PACK=1, 3T instead of 6T cycles per chunk).
"""

import math

import numpy as np

import concourse.bass as bass  # noqa: F401
import concourse.mybir as mybir
from concourse import bacc
from concourse.bass_utils import run_bass_kernel_spmd
from concourse.tile import TileContext

F32 = mybir.dt.float32
F16 = mybir.dt.float16
MM_DT = F16
MM_NP = np.float16

N_CORES = 8
N_SLOTS = 3
D = 768
H = 768
C = 2
DT = D // 128  # 6 d-tiles
HT = H // 128  # 6 h-tiles
MAX_CHUNK = 512  # one PSUM bank holds 512 fp32 -> matmul free dim cap
FIRST_CHUNK = 256  # small first chunk -> early PE start

L2_PACK = True
L2_M = 32  # pad W2's C=2 cols to a full 32-wide column group
N_WARMUP = 8  # fp16 warmup matmuls to ramp the PE clock during DMA fill


def _chunk_list(cap: int, slot: int) -> list[int]:
    """Chunk sizes for a slot's cap. Slot 0 leads with a small chunk for
    an early PE start; the last slot ends with a 128 tail so only a short
    relu->L2->add->DMA chain trails the final big matmul."""
    if slot == 0 and cap > FIRST_CHUNK + 128:
        rest = cap - FIRST_CHUNK
        n = max(1, math.ceil(rest / MAX_CHUNK))
        while rest % n or (rest // n) % 2:
            n += 1
        sizes = [FIRST_CHUNK] + [rest // n] * n
    else:
        n = max(1, math.ceil(cap / MAX_CHUNK))
        while cap % n or (cap // n) % 2:
            n += 1
        sizes = [cap // n] * n
    if slot == N_SLOTS - 1 and sizes[-1] >= 384:
        sizes[-1:] = [sizes[-1] - 128, 128]
    return sizes


def _round_cap(cap: int) -> int:
    """Round capacity up so it splits into equal, even chunks <= 512."""
    cap = max(cap, 256)
    n = max(1, math.ceil(cap / MAX_CHUNK))
    return 2 * n * math.ceil(cap / (2 * n))


def _plan_packing(counts: np.ndarray):
    """Split the largest expert groups in half until there are 24 pieces,
    sort descending, deal 8 pieces per slot group; caps = per-group max.
    (This minimizes sum-of-group-maxima: the k-th group's cap is bounded
    below by the (8k+1)-th largest piece.)
    Returns (caps, assign): assign[s][c]=(expert,start,len)."""
    frags = [(int(e), 0, int(c)) for e, c in enumerate(counts) if c > 0]
    target = N_CORES * N_SLOTS
    assert len(frags) <= target, (
        f"{len(frags)} non-empty experts exceed {target} slots"
    )
    while len(frags) < target:
        frags.sort(key=lambda f: -f[2])
        e, st, ln = frags[0]
        if ln < 2:
            frags.append((e, st, 0))
            continue
        h1 = ln // 2
        frags[0] = (e, st, ln - h1)
        frags.append((e, st + (ln - h1), h1))
    frags.sort(key=lambda f: -f[2])
    caps, assign = [], []
    for s in range(N_SLOTS):
        group = frags[s * N_CORES : (s + 1) * N_CORES]
        caps.append(_round_cap(max(f[2] for f in group)))
        assign.append(group)
    return caps, assign


_PROGRAM_CACHE: dict = {}


def _build_program(caps: tuple):
    if caps in _PROGRAM_CACHE:
        return _PROGRAM_CACHE[caps]

    R = sum(caps)
    chunk_lists = [_chunk_list(caps[s], s) for s in range(N_SLOTS)]
    CTOT = DT * R

    nc = bacc.Bacc(
        "TRN2", target_bir_lowering=False, debug=False, num_devices=N_CORES
    )
    xT = nc.dram_tensor("xT", [128, CTOT], MM_DT, kind="ExternalInput")
    w1 = nc.dram_tensor("w1", [N_SLOTS, DT, 128, H], MM_DT, kind="ExternalInput")
    b1 = nc.dram_tensor("b1", [N_SLOTS, 128, HT], F32, kind="ExternalInput")
    w2_m = L2_M if L2_PACK else C
    w2 = nc.dram_tensor("w2", [N_SLOTS, 128, HT, w2_m], MM_DT, kind="ExternalInput")
    if L2_PACK:
        sel = nc.dram_tensor("sel", [128, C], MM_DT, kind="ExternalInput")
    b2 = nc.dram_tensor("b2", [N_SLOTS, C, 1], F32, kind="ExternalInput")
    y = nc.dram_tensor("y", [C, R], F32, kind="ExternalOutput")

    with TileContext(nc) as tc:
        with (
            tc.tile_pool(name="wpool", bufs=1) as wpool,
            tc.tile_pool(name="xpool", bufs=1) as xpool,
            tc.tile_pool(name="hpool", bufs=4) as hpool,
            tc.tile_pool(name="ypool", bufs=1) as ypool,
            tc.tile_pool(name="pspool", bufs=6, space="PSUM") as pspool,
            tc.tile_pool(name="pypool", bufs=1, space="PSUM") as pypool,
        ):
            y_sb = ypool.tile([C, R], F32, name="y_sb")

            # PE warm-up: ramps the HAM clock gate while DMAs fill. fp16 so
            # each is a single short pass; queued before real LDWEIGHTS.
            wu_w = ypool.tile([128, 128], MM_DT, name="wu_w")
            wu_x = ypool.tile([128, 256], MM_DT, name="wu_x")
            nc.gpsimd.memset(wu_w[:, :], 0.0)
            nc.gpsimd.memset(wu_x[:, :], 0.0)
            wu_ps = pspool.tile([128, 512], F32, name="wu_ps", tag="psh")
            for _ in range(N_WARMUP):
                nc.tensor.matmul(
                    wu_ps[:, :256], wu_w, wu_x, start=True, stop=True
                )

            # ---- tiles ----
            xs_t = []  # xs_t[s][ci] = [128, DT, size]
            off = 0
            xoffs = []
            for s in range(N_SLOTS):
                xs_t.append([])
                xoffs.append([])
                for ci, size in enumerate(chunk_lists[s]):
                    xs_t[s].append(
                        xpool.tile(
                            [128, DT, size], MM_DT, name=f"xs{s}_{ci}",
                            tag=f"xs{s}_{ci}",
                        )
                    )
                    xoffs[s].append(off)
                    off += DT * size
            w1_t = [
                [
                    wpool.tile([128, H], MM_DT, name=f"w1_{s}_{dt}",
                               tag=f"w1_{s}_{dt}")
                    for dt in range(DT)
                ]
                for s in range(N_SLOTS)
            ]
            b1_t = [
                wpool.tile([128, HT], F32, name=f"b1_{s}", tag=f"b1_{s}")
                for s in range(N_SLOTS)
            ]
            w2_t = [
                wpool.tile([128, HT, w2_m], MM_DT, name=f"w2_{s}",
                           tag=f"w2_{s}")
                for s in range(N_SLOTS)
            ]
            b2_t = [
                wpool.tile([C, 1], F32, name=f"b2_{s}", tag=f"b2_{s}")
                for s in range(N_SLOTS)
            ]
            if L2_PACK:
                sel_sb = ypool.tile([128, C], MM_DT, name="sel_sb")

            # ---- DMA issue schedule ----
            # scalar queue: slot-0 x chunks + slot-0 smalls (scalar is free
            # until the first relu at ~13us; each issue costs ~0.67us)
            for ci in range(len(chunk_lists[0])):
                o = xoffs[0][ci]
                nc.scalar.dma_start(
                    out=xs_t[0][ci],
                    in_=xT[:, o : o + DT * chunk_lists[0][ci]],
                )
            nc.scalar.dma_start(out=b1_t[0], in_=b1[0])
            nc.scalar.dma_start(out=w2_t[0], in_=w2[0])
            nc.scalar.dma_start(out=b2_t[0], in_=b2[0])
            if L2_PACK:
                nc.scalar.dma_start(out=sel_sb[:, :], in_=sel[:, :])
            # sync queue: slot-0 weights first (needed with first chunk),
            # then the rest slot by slot
            for dt in range(DT):
                nc.sync.dma_start(out=w1_t[0][dt], in_=w1[0, dt])
            for s in range(1, N_SLOTS):
                for ci, size in enumerate(chunk_lists[s]):
                    o = xoffs[s][ci]
                    nc.sync.dma_start(
                        out=xs_t[s][ci], in_=xT[:, o : o + DT * size]
                    )
                for dt in range(DT):
                    nc.sync.dma_start(out=w1_t[s][dt], in_=w1[s, dt])
                nc.sync.dma_start(out=b1_t[s], in_=b1[s])
                nc.sync.dma_start(out=w2_t[s], in_=w2[s])
                nc.sync.dma_start(out=b2_t[s], in_=b2[s])

            # ---- compute ----
            off = 0
            for s in range(N_SLOTS):
                co = 0
                for ci, size in enumerate(chunk_lists[s]):
                    xt = xs_t[s][ci]
                    h_sb = hpool.tile([128, HT, size], MM_DT, name="h_sb",
                                      tag="h")
                    if s == 0 and ci == 0:
                        # dt-major: round dt needs only w1_d{dt} + this
                        # chunk's x -> PE starts on the first 0.6MB
                        ps6 = [
                            pspool.tile([128, size], F32, name=f"ps_h{ht}",
                                        tag="psh")
                            for ht in range(HT)
                        ]
                        for dt in range(DT):
                            for ht in range(HT):
                                nc.tensor.matmul(
                                    ps6[ht],
                                    w1_t[s][dt][:, ht * 128 : (ht + 1) * 128],
                                    xt[:, dt, :],
                                    start=(dt == 0),
                                    stop=(dt == DT - 1),
                                )
                        for ht in range(HT):
                            nc.scalar.activation(
                                h_sb[:, ht, :],
                                ps6[ht],
                                mybir.ActivationFunctionType.Relu,
                                bias=b1_t[s][:, ht : ht + 1],
                            )
                    else:
                        for ht in range(HT):
                            ps_h = pspool.tile([128, size], F32, name="ps_h",
                                               tag="psh")
                            for dt in range(DT):
                                nc.tensor.matmul(
                                    ps_h,
                                    w1_t[s][dt][:, ht * 128 : (ht + 1) * 128],
                                    xt[:, dt, :],
                                    start=(dt == 0),
                                    stop=(dt == DT - 1),
                                )
                            nc.scalar.activation(
                                h_sb[:, ht, :],
                                ps_h,
                                mybir.ActivationFunctionType.Relu,
                                bias=b1_t[s][:, ht : ht + 1],
                            )
                    if L2_PACK:
                        # 4 concurrent PE column groups (2 rounds), then a
                        # selector matmul folds the partials into [C, T]
                        ps_y4 = pypool.tile([128, size], F32, name="ps_y4",
                                            tag="psy")
                        for ht in range(HT):
                            g = ht % 4
                            nc.tensor.matmul(
                                ps_y4[32 * g : 32 * g + L2_M, :],
                                w2_t[s][:, ht, :],
                                h_sb[:, ht, :],
                                start=(ht < 4),
                                stop=(ht >= 4 or g >= HT - 4),
                                tile_position=(0, 32 * g),
                            )
                        y4_sb = hpool.tile([128, size], MM_DT, name="y4_sb",
                                           tag="y4")
                        nc.scalar.activation(
                            y4_sb, ps_y4, mybir.ActivationFunctionType.Copy
                        )
                        ps_y = pypool.tile([C, size], F32, name="ps_y",
                                           tag="psy2")
                        nc.tensor.matmul(ps_y, sel_sb, y4_sb, start=True,
                                         stop=True)
                    else:
                        ps_y = pypool.tile([C, size], F32, name="ps_y",
                                           tag="psy2")
                        for ht in range(HT):
                            nc.tensor.matmul(
                                ps_y,
                                w2_t[s][:, ht, :],
                                h_sb[:, ht, :],
                                start=(ht == 0),
                                stop=(ht == HT - 1),
                            )
                    nc.vector.tensor_scalar_add(
                        y_sb[:, off + co : off + co + size], ps_y, b2_t[s][:, :]
                    )
                    if s == N_SLOTS - 1:
                        nc.sync.dma_start(
                            out=y[:, off + co : off + co + size],
                            in_=y_sb[:, off + co : off + co + size],
                        )
                    co += size
                if s != N_SLOTS - 1:
                    nc.sync.dma_start(
                        out=y[:, off : off + caps[s]],
                        in_=y_sb[:, off : off + caps[s]],
                    )
                off += caps[s]
    nc.compile()
    _PROGRAM_CACHE[caps] = nc
    return nc


def kernel(embeddings, component_idx, W1, b1, W2, b2):
    embeddings = np.ascontiguousarray(np.asarray(embeddings, dtype=np.float32))
    ci = np.asarray(component_idx).astype(np.int64, copy=False)
    W1 = np.asarray(W1, dtype=np.float32)
    b1 = np.asarray(b1, dtype=np.float32)
    W2 = np.asarray(W2, dtype=np.float32)
    b2 = np.asarray(b2, dtype=np.float32)

    N = embeddings.shape[0]
    E = W1.shape[0]

    counts = np.bincount(ci, minlength=E)
    order = np.argsort(ci, kind="stable")
    group_start = np.zeros(E, dtype=np.int64)
    group_start[1:] = np.cumsum(counts)[:-1]
    x_sorted = embeddings[order]  # [N, D] grouped by expert

    caps, assign = _plan_packing(counts)
    R = sum(caps)
    offs = np.cumsum([0] + caps[:-1]).tolist()
    chunk_lists = [_chunk_list(caps[s], s) for s in range(N_SLOTS)]

    nc = _build_program(tuple(caps))

    # host-side packing of per-core inputs
    w1_packed = W1.reshape(E, DT, 128, H).astype(MM_NP)  # [e, dt, din, h]
    b1_packed = np.ascontiguousarray(
        b1.reshape(E, HT, 128).transpose(0, 2, 1)
    )  # [e, 128, ht]
    w2_m = L2_M if L2_PACK else C
    w2_packed = np.zeros((E, 128, HT, w2_m), dtype=MM_NP)
    w2_packed[:, :, :, :C] = W2.reshape(E, HT, 128, C).transpose(0, 2, 1, 3)
    b2_packed = b2.reshape(E, C, 1)

    CTOT = DT * R
    in_maps = []
    for c in range(N_CORES):
        Xc = np.zeros((R, D), dtype=np.float32)
        w1_in = np.empty((N_SLOTS, DT, 128, H), dtype=MM_NP)
        b1_in = np.empty((N_SLOTS, 128, HT), dtype=np.float32)
        w2_in = np.empty((N_SLOTS, 128, HT, w2_m), dtype=MM_NP)
        b2_in = np.empty((N_SLOTS, C, 1), dtype=np.float32)
        for s in range(N_SLOTS):
            e, st, ln = assign[s][c]
            beg = group_start[e] + st
            Xc[offs[s] : offs[s] + ln] = x_sorted[beg : beg + ln]
            w1_in[s] = w1_packed[e]
            b1_in[s] = b1_packed[e]
            w2_in[s] = w2_packed[e]
            b2_in[s] = b2_packed[e]
        # per-chunk layout: [128, DT, sz] blocks, concatenated
        xT_in = np.empty((128, CTOT), dtype=MM_NP)
        xoff = 0
        tok = 0
        for s in range(N_SLOTS):
            for sz in chunk_lists[s]:
                blk = Xc[tok : tok + sz].T.astype(MM_NP)  # [768, sz]
                xT_in[:, xoff : xoff + DT * sz] = (
                    blk.reshape(DT, 128, sz).transpose(1, 0, 2).reshape(128, -1)
                )
                xoff += DT * sz
                tok += sz
        im = {"xT": xT_in, "w1": w1_in, "b1": b1_in, "w2": w2_in, "b2": b2_in}
        if L2_PACK:
            sel_np = np.zeros((128, C), dtype=MM_NP)
            for g in range(4):
                for cc in range(C):
                    sel_np[32 * g + cc, cc] = 1
            im["sel"] = sel_np
        in_maps.append(im)

    global _LAST_IN_MAPS
    _LAST_IN_MAPS = in_maps
    res = run_bass_kernel_spmd(nc, in_maps, list(range(N_CORES)))

    out = np.empty((N, C), dtype=np.float32)
    for c in range(N_CORES):
        yc = res.results[c]["y"]  # [C, R]
        for s in range(N_SLOTS):
            e, st, ln = assign[s][c]
            if ln == 0:
                continue
            beg = group_start[e] + st
            tokens = order[beg : beg + ln]
            out[tokens] = yc[:, offs[s] : offs[s] + ln].T
    return out


# revision 12
# speedup vs baseline: 1.1575x; 1.1575x over previous
"""MoE routed-expert kernel for Trainium2 (8 NeuronCores, SPMD).

Problem: N=16384 tokens, D=768, H=768, C=2, E=20 experts.
  y[n] = relu(x[n] @ W1[e] + b1[e]) @ W2[e] + b2[e],  e = component_idx[n]

Strategy
--------
Host side (numpy): sort tokens by expert; search cap triples (c0,c1,c2)
minimizing padded tokens R=c0+c1+c2 such that the 20 expert groups can be
split into <=24 pieces that fill 8 cores x 3 "expert slots" (slot s holds
c_s tokens of ONE expert per core).  Every core runs the SAME static
program (SPMD); which expert a slot holds is purely which weights/tokens
the host stages into that core's buffers.

Device side (Bass/Tile, per core): x is staged per chunk as [128, DT*sz]
(d-tiles contiguous) so ONE DMA feeds a whole chunk.  DMA issues are split
across the Sync and Scalar queues (descriptor issue costs ~0.6us each on
one queue).  The first chunk is small (256 tokens) and runs dt-major so
the PE starts as soon as w1_d0 + one 0.4MB chunk land.  Layer 1: 6x6
accumulating fp16 matmuls per chunk; relu+bias fused on ScalarE.  Layer 2:
either 6 plain [128,2] matmuls (L2_PACK=0) or 4-way PE column groups +
selector reduction (L2_PACK=1, 3T instead of 6T cycles per chunk).
"""

import math

import numpy as np

import concourse.bass as bass  # noqa: F401
import concourse.mybir as mybir
from concourse import bacc
from concourse.bass_utils import run_bass_kernel_spmd
from concourse.tile import TileContext

F32 = mybir.dt.float32
F16 = mybir.dt.float16
MM_DT = F16
MM_NP = np.float16

N_CORES = 8
N_SLOTS = 3
D = 768
H = 768
C = 2
DT = D // 128  # 6 d-tiles
HT = H // 128  # 6 h-tiles
MAX_CHUNK = 512  # one PSUM bank holds 512 fp32 -> matmul free dim cap
FIRST_CHUNK = 256  # small first chunk -> early PE start

L2_PACK = True
L2_M = 32  # pad W2's C=2 cols to a full 32-wide column group
N_WARMUP = 8  # fp16 warmup matmuls to ramp the PE clock during DMA fill


def _chunk_list(cap: int, slot: int) -> list[int]:
    """Chunk sizes for a slot's cap. Slot 0 leads with a small chunk for
    an early PE start; the last slot ends with a 128 tail so only a short
    relu->L2->add->DMA chain trails the final big matmul."""
    if slot == 0 and cap > FIRST_CHUNK + 128:
        rest = cap - FIRST_CHUNK
        n = max(1, math.ceil(rest / MAX_CHUNK))
        while rest % n or (rest // n) % 2:
            n += 1
        sizes = [FIRST_CHUNK] + [rest // n] * n
    else:
        n = max(1, math.ceil(cap / MAX_CHUNK))
        while cap % n or (cap // n) % 2:
            n += 1
        sizes = [cap // n] * n
    if slot == N_SLOTS - 1 and sizes[-1] >= 384:
        sizes[-1:] = [sizes[-1] - 128, 128]
    return sizes


def _round_cap(cap: int) -> int:
    """Round capacity up so it splits into equal, even chunks <= 512."""
    cap = max(cap, 256)
    n = max(1, math.ceil(cap / MAX_CHUNK))
    return 2 * n * math.ceil(cap / (2 * n))


def _plan_packing(counts: np.ndarray):
    """Split the largest expert groups in half until there are 24 pieces,
    sort descending, deal 8 pieces per slot group; caps = per-group max.
    (This minimizes sum-of-group-maxima: the k-th group's cap is bounded
    below by the (8k+1)-th largest piece.)
    Returns (caps, assign): assign[s][c]=(expert,start,len)."""
    frags = [(int(e), 0, int(c)) for e, c in enumerate(counts) if c > 0]
    target = N_CORES * N_SLOTS
    assert len(frags) <= target, (
        f"{len(frags)} non-empty experts exceed {target} slots"
    )
    while len(frags) < target:
        frags.sort(key=lambda f: -f[2])
        e, st, ln = frags[0]
        if ln < 2:
            frags.append((e, st, 0))
            continue
        h1 = ln // 2
        frags[0] = (e, st, ln - h1)
        frags.append((e, st + (ln - h1), h1))
    frags.sort(key=lambda f: -f[2])
    caps, assign = [], []
    for s in range(N_SLOTS):
        group = frags[s * N_CORES : (s + 1) * N_CORES]
        caps.append(_round_cap(max(f[2] for f in group)))
        assign.append(group)
    return caps, assign


_PROGRAM_CACHE: dict = {}


def _build_program(caps: tuple):
    if caps in _PROGRAM_CACHE:
        return _PROGRAM_CACHE[caps]

    R = sum(caps)
    chunk_lists = [_chunk_list(caps[s], s) for s in range(N_SLOTS)]
    CTOT = DT * R

    nc = bacc.Bacc(
        "TRN2", target_bir_lowering=False, debug=False, num_devices=N_CORES
    )
    xT = nc.dram_tensor("xT", [128, CTOT], MM_DT, kind="ExternalInput")
    w1 = nc.dram_tensor("w1", [N_SLOTS, DT, 128, H], MM_DT, kind="ExternalInput")
    b1 = nc.dram_tensor("b1", [N_SLOTS, 128, HT], F32, kind="ExternalInput")
    w2_m = L2_M if L2_PACK else C
    w2 = nc.dram_tensor("w2", [N_SLOTS, 128, HT, w2_m], MM_DT, kind="ExternalInput")
    if L2_PACK:
        sel = nc.dram_tensor("sel", [128, C], MM_DT, kind="ExternalInput")
    b2 = nc.dram_tensor("b2", [N_SLOTS, C, 1], F32, kind="ExternalInput")
    y = nc.dram_tensor("y", [C, R], F32, kind="ExternalOutput")

    with TileContext(nc) as tc:
        with (
            tc.tile_pool(name="wpool", bufs=1) as wpool,
            tc.tile_pool(name="xpool", bufs=1) as xpool,
            tc.tile_pool(name="hpool", bufs=4) as hpool,
            tc.tile_pool(name="ypool", bufs=1) as ypool,
            tc.tile_pool(name="pspool", bufs=6, space="PSUM") as pspool,
            tc.tile_pool(name="pypool", bufs=2, space="PSUM") as pypool,
        ):
            y_sb = ypool.tile([C, R], F32, name="y_sb")

            # PE warm-up: ramps the HAM clock gate while DMAs fill. fp32
            # LOW_HIGH pairs are slow at ramp speed, bridging the ~4us until
            # the first chunk's data lands without a PE gap.
            wu_w = ypool.tile([128, 128], F32, name="wu_w")
            wu_x = ypool.tile([128, 256], F32, name="wu_x")
            nc.gpsimd.memset(wu_w[:, :], 0.0)
            nc.gpsimd.memset(wu_x[:, :], 0.0)
            wu_ps = pspool.tile([128, 512], F32, name="wu_ps", tag="psh")
            for _ in range(4):
                nc.tensor.matmul(
                    wu_ps[:, :256], wu_w, wu_x, start=True, stop=True
                )

            # ---- tiles ----
            xs_t = []  # xs_t[s][ci] = [128, DT, size]
            off = 0
            xoffs = []
            for s in range(N_SLOTS):
                xs_t.append([])
                xoffs.append([])
                for ci, size in enumerate(chunk_lists[s]):
                    xs_t[s].append(
                        xpool.tile(
                            [128, DT, size], MM_DT, name=f"xs{s}_{ci}",
                            tag=f"xs{s}_{ci}",
                        )
                    )
                    xoffs[s].append(off)
                    off += DT * size
            w1_t = [
                [
                    wpool.tile([128, H], MM_DT, name=f"w1_{s}_{dt}",
                               tag=f"w1_{s}_{dt}")
                    for dt in range(DT)
                ]
                for s in range(N_SLOTS)
            ]
            b1_t = [
                wpool.tile([128, HT], F32, name=f"b1_{s}", tag=f"b1_{s}")
                for s in range(N_SLOTS)
            ]
            w2_t = [
                wpool.tile([128, HT, w2_m], MM_DT, name=f"w2_{s}",
                           tag=f"w2_{s}")
                for s in range(N_SLOTS)
            ]
            b2_t = [
                wpool.tile([C, 1], F32, name=f"b2_{s}", tag=f"b2_{s}")
                for s in range(N_SLOTS)
            ]
            if L2_PACK:
                sel_sb = ypool.tile([128, C], MM_DT, name="sel_sb")

            # ---- DMA issue schedule ----
            # Single sync HWDGE queue (a second queue contends on the shared
            # HWDGE block and adds setup time). Critical-first order: the
            # first chunk's x + slot-0 weights + b1 before anything else.
            def load_x(s, ci):
                o = xoffs[s][ci]
                nc.sync.dma_start(
                    out=xs_t[s][ci],
                    in_=xT[:, o : o + DT * chunk_lists[s][ci]],
                )

            load_x(0, 0)
            for dt in range(DT):
                nc.sync.dma_start(out=w1_t[0][dt], in_=w1[0, dt])
            nc.sync.dma_start(out=b1_t[0], in_=b1[0])
            for ci in range(1, len(chunk_lists[0])):
                load_x(0, ci)
            nc.sync.dma_start(out=w2_t[0], in_=w2[0])
            nc.sync.dma_start(out=b2_t[0], in_=b2[0])
            if L2_PACK:
                nc.sync.dma_start(out=sel_sb[:, :], in_=sel[:, :])
            for s in range(1, N_SLOTS):
                for ci in range(len(chunk_lists[s])):
                    load_x(s, ci)
                for dt in range(DT):
                    nc.sync.dma_start(out=w1_t[s][dt], in_=w1[s, dt])
                nc.sync.dma_start(out=b1_t[s], in_=b1[s])
                nc.sync.dma_start(out=w2_t[s], in_=w2[s])
                nc.sync.dma_start(out=b2_t[s], in_=b2[s])

            # ---- compute: software-pipelined across chunks ----
            # PE stream per chunk k:  [L1(k) 36mm] [L2grp(k-1) 6mm]
            # [sel(k-2) 1mm] -- every stage's inputs were produced a full
            # chunk earlier, so the in-order PE queue never waits on the
            # relu/copy chains.
            slot_offs = np.cumsum([0] + list(caps[:-1])).tolist()
            chunks = []  # (slot, size, y_off, last_of_slot)
            for s in range(N_SLOTS):
                co = 0
                for ci, size in enumerate(chunk_lists[s]):
                    chunks.append(
                        (s, size, slot_offs[s] + co,
                         ci == len(chunk_lists[s]) - 1)
                    )
                    co += size
            state = {}  # k -> dict(h_sb=..., ps_y4=..., y4_sb=..., ps_y=...)

            def emit_l1(k, dt_major):
                s, size, _, _ = chunks[k]
                ci = k - sum(len(chunk_lists[t]) for t in range(s))
                xt = xs_t[s][ci]
                h_sb = hpool.tile([128, HT, size], MM_DT, name="h_sb",
                                  tag="h")
                if dt_major:
                    ps6 = [
                        pspool.tile([128, size], F32, name=f"ps_h{ht}",
                                    tag="psh")
                        for ht in range(HT)
                    ]
                    for dt in range(DT):
                        for ht in range(HT):
                            nc.tensor.matmul(
                                ps6[ht],
                                w1_t[s][dt][:, ht * 128 : (ht + 1) * 128],
                                xt[:, dt, :],
                                start=(dt == 0),
                                stop=(dt == DT - 1),
                            )
                    for ht in range(HT):
                        nc.scalar.activation(
                            h_sb[:, ht, :],
                            ps6[ht],
                            mybir.ActivationFunctionType.Relu,
                            bias=b1_t[s][:, ht : ht + 1],
                        )
                else:
                    for ht in range(HT):
                        ps_h = pspool.tile([128, size], F32, name="ps_h",
                                           tag="psh")
                        for dt in range(DT):
                            nc.tensor.matmul(
                                ps_h,
                                w1_t[s][dt][:, ht * 128 : (ht + 1) * 128],
                                xt[:, dt, :],
                                start=(dt == 0),
                                stop=(dt == DT - 1),
                            )
                        nc.scalar.activation(
                            h_sb[:, ht, :],
                            ps_h,
                            mybir.ActivationFunctionType.Relu,
                            bias=b1_t[s][:, ht : ht + 1],
                        )
                state[k] = {"h_sb": h_sb}

            def emit_grp(k):
                """Layer-2 matmuls for chunk k; with L2_PACK, 4 column
                groups run concurrently (2 rounds) + a scalar copy to SBUF
                for the later selector matmul."""
                s, size, _, _ = chunks[k]
                st = state[k]
                h_sb = st["h_sb"]
                if L2_PACK:
                    ps_y4 = pypool.tile([128, size], F32, name="ps_y4",
                                        tag="psy")
                    for ht in range(HT):
                        g = ht % 4
                        nc.tensor.matmul(
                            ps_y4[32 * g : 32 * g + L2_M, :],
                            w2_t[s][:, ht, :],
                            h_sb[:, ht, :],
                            start=(ht < 4),
                            stop=(ht >= 4 or g >= HT - 4),
                            tile_position=(0, 32 * g),
                        )
                    y4_sb = hpool.tile([128, size], MM_DT, name="y4_sb",
                                       tag="y4")
                    nc.scalar.activation(
                        y4_sb, ps_y4, mybir.ActivationFunctionType.Copy
                    )
                    st["y4_sb"] = y4_sb
                else:
                    ps_y = pypool.tile([C, size], F32, name="ps_y",
                                       tag="psy")
                    for ht in range(HT):
                        nc.tensor.matmul(
                            ps_y,
                            w2_t[s][:, ht, :],
                            h_sb[:, ht, :],
                            start=(ht == 0),
                            stop=(ht == HT - 1),
                        )
                    st["ps_y"] = ps_y

            def emit_sel_add(k):
                s, size, y_off, last = chunks[k]
                st = state[k]
                if L2_PACK:
                    ps_y = pypool.tile([C, size], F32, name="ps_y",
                                       tag="psy")
                    nc.tensor.matmul(ps_y, sel_sb, st["y4_sb"], start=True,
                                     stop=True)
                else:
                    ps_y = st["ps_y"]
                nc.vector.tensor_scalar_add(
                    y_sb[:, y_off : y_off + size], ps_y, b2_t[s][:, :]
                )
                if s == N_SLOTS - 1:
                    nc.sync.dma_start(
                        out=y[:, y_off : y_off + size],
                        in_=y_sb[:, y_off : y_off + size],
                    )
                elif last:
                    nc.sync.dma_start(
                        out=y[:, slot_offs[s] : slot_offs[s] + caps[s]],
                        in_=y_sb[:, slot_offs[s] : slot_offs[s] + caps[s]],
                    )
                del state[k]

            nchunks = len(chunks)
            for k in range(nchunks):
                emit_l1(k, dt_major=(k == 0))
                if k >= 1:
                    emit_grp(k - 1)
                if k >= 2:
                    emit_sel_add(k - 2)
            emit_grp(nchunks - 1)
            if nchunks >= 2:
                emit_sel_add(nchunks - 2)
            emit_sel_add(nchunks - 1)
    nc.compile()
    _PROGRAM_CACHE[caps] = nc
    return nc


def kernel(embeddings, component_idx, W1, b1, W2, b2):
    embeddings = np.ascontiguousarray(np.asarray(embeddings, dtype=np.float32))
    ci = np.asarray(component_idx).astype(np.int64, copy=False)
    W1 = np.asarray(W1, dtype=np.float32)
    b1 = np.asarray(b1, dtype=np.float32)
    W2 = np.asarray(W2, dtype=np.float32)
    b2 = np.asarray(b2, dtype=np.float32)

    N = embeddings.shape[0]
    E = W1.shape[0]

    counts = np.bincount(ci, minlength=E)
    order = np.argsort(ci, kind="stable")
    group_start = np.zeros(E, dtype=np.int64)
    group_start[1:] = np.cumsum(counts)[:-1]
    x_sorted = embeddings[order]  # [N, D] grouped by expert

    caps, assign = _plan_packing(counts)
    R = sum(caps)
    offs = np.cumsum([0] + caps[:-1]).tolist()
    chunk_lists = [_chunk_list(caps[s], s) for s in range(N_SLOTS)]

    nc = _build_program(tuple(caps))

    # host-side packing of per-core inputs
    w1_packed = W1.reshape(E, DT, 128, H).astype(MM_NP)  # [e, dt, din, h]
    b1_packed = np.ascontiguousarray(
        b1.reshape(E, HT, 128).transpose(0, 2, 1)
    )  # [e, 128, ht]
    w2_m = L2_M if L2_PACK else C
    w2_packed = np.zeros((E, 128, HT, w2_m), dtype=MM_NP)
    w2_packed[:, :, :, :C] = W2.reshape(E, HT, 128, C).transpose(0, 2, 1, 3)
    b2_packed = b2.reshape(E, C, 1)

    CTOT = DT * R
    in_maps = []
    for c in range(N_CORES):
        Xc = np.zeros((R, D), dtype=np.float32)
        w1_in = np.empty((N_SLOTS, DT, 128, H), dtype=MM_NP)
        b1_in = np.empty((N_SLOTS, 128, HT), dtype=np.float32)
        w2_in = np.empty((N_SLOTS, 128, HT, w2_m), dtype=MM_NP)
        b2_in = np.empty((N_SLOTS, C, 1), dtype=np.float32)
        for s in range(N_SLOTS):
            e, st, ln = assign[s][c]
            beg = group_start[e] + st
            Xc[offs[s] : offs[s] + ln] = x_sorted[beg : beg + ln]
            w1_in[s] = w1_packed[e]
            b1_in[s] = b1_packed[e]
            w2_in[s] = w2_packed[e]
            b2_in[s] = b2_packed[e]
        # per-chunk layout: [128, DT, sz] blocks, concatenated
        xT_in = np.empty((128, CTOT), dtype=MM_NP)
        xoff = 0
        tok = 0
        for s in range(N_SLOTS):
            for sz in chunk_lists[s]:
                blk = Xc[tok : tok + sz].T.astype(MM_NP)  # [768, sz]
                xT_in[:, xoff : xoff + DT * sz] = (
                    blk.reshape(DT, 128, sz).transpose(1, 0, 2).reshape(128, -1)
                )
                xoff += DT * sz
                tok += sz
        im = {"xT": xT_in, "w1": w1_in, "b1": b1_in, "w2": w2_in, "b2": b2_in}
        if L2_PACK:
            sel_np = np.zeros((128, C), dtype=MM_NP)
            for g in range(4):
                for cc in range(C):
                    sel_np[32 * g + cc, cc] = 1
            im["sel"] = sel_np
        in_maps.append(im)

    global _LAST_IN_MAPS
    _LAST_IN_MAPS = in_maps
    res = run_bass_kernel_spmd(nc, in_maps, list(range(N_CORES)))

    out = np.empty((N, C), dtype=np.float32)
    for c in range(N_CORES):
        yc = res.results[c]["y"]  # [C, R]
        for s in range(N_SLOTS):
            e, st, ln = assign[s][c]
            if ln == 0:
                continue
            beg = group_start[e] + st
            tokens = order[beg : beg + ln]
            out[tokens] = yc[:, offs[s] : offs[s] + ln].T
    return out
